# revision 16
# baseline (speedup 1.0000x reference)
"""Trainium2 Bass kernel for causal AttentionBlock.

Reference computation (per batch b):
    K = x @ Wk + bk ; Q = x @ Wq + bq ; V = x @ Wv + bv      # [T, 512]
    logits = Q @ K^T  (causal: allow s <= t)
    probs  = softmax(logits) / sqrt(512)
    read   = probs @ V
    out    = concat([x, read], axis=-1)                      # [T, 1536]

Shapes: B=4, T=2048, D=1024, K=V=512, all float32.

Sharding over 8 cores: core = 2*b + c where b = batch, c = query parity.
Core (b, c) owns query strips s = 2j + c (j = 0..7), 128 rows each —
interleaved strips balance causal work exactly: the number of 512-wide key
tiles needed per own-strip j is NK[j] = [1,1,2,2,3,3,4,4] for BOTH parities,
so a single SPMD program serves all cores.  Per-core differences (which
rows are queries, causal boundary masks) are carried in the input data.

Each core computes K^T and V over the full T (duplicated within the batch
pair), Q^T for its own 1024 rows, then causally-truncated attention.
The sqrt(512) quirk and bv are folded into the output stage:
    read = (exp @ V0) * rinv + bv/sqrt(512),  rinv = 1/(rowsum * sqrt(512))
because softmax rows sum to 1 (so the bv term picks up exactly 1/sqrt(512)).
"""

import os
os.environ.setdefault("JAX_COMPILATION_CACHE_DIR", "/tmp/jaxcache")

import numpy as np

import concourse.bass as bass
import concourse.tile as tile
from concourse import mybir
from concourse.bass_utils import run_bass_kernel_spmd
from concourse.masks import make_identity
from concourse.tile import add_dep_helper

P = 128
B, T, D, F = 4, 2048, 1024, 512
DC = D // P        # 8 contraction chunks
FC = F // P        # 4 feature chunks
NSTRIP = T // P    # 16 strips per batch
NOWN = NSTRIP // 2  # 8 own strips per core
NK = [1, 1, 2, 2, 3, 3, 4, 4]   # 512-wide key tiles per own strip (both parities)
SQRT_K = float(np.sqrt(512.0))
NEG = -1.0e30

f32 = mybir.dt.float32

last_result = None  # BassKernelResults of the most recent run (for test.py)


def _build_bass() -> bass.Bass:
    nc = bass.Bass()

    xT = nc.dram_tensor("xt", [DC, P, T], f32, kind="ExternalInput")
    xqT = nc.dram_tensor("xqt", [DC, P, NOWN * P], f32, kind="ExternalInput")
    wk_d = nc.dram_tensor("wk", [DC, P, F], f32, kind="ExternalInput")
    wq_d = nc.dram_tensor("wq", [DC, P, F], f32, kind="ExternalInput")
    wv_d = nc.dram_tensor("wv", [DC, P, F], f32, kind="ExternalInput")
    bkt_d = nc.dram_tensor("bkt", [P, FC], f32, kind="ExternalInput")
    bqt_d = nc.dram_tensor("bqt", [P, FC], f32, kind="ExternalInput")
    bvb_d = nc.dram_tensor("bvb", [1, F], f32, kind="ExternalInput")
    mask_d = nc.dram_tensor("mask", [NOWN, P, F], f32, kind="ExternalInput")
    out_d = nc.dram_tensor("out", [NOWN, P, F], f32, kind="ExternalOutput")

    with tile.TileContext(nc) as tc:
        with (
            tc.tile_pool(name="singles", bufs=1) as singles,
            tc.tile_pool(name="xin", bufs=2) as xin,
            tc.tile_pool(name="work", bufs=2) as work,
            tc.tile_pool(name="small", bufs=4) as small,
            tc.tile_pool(name="psum", bufs=7, space="PSUM") as psum,
            tc.tile_pool(name="psum1", bufs=1, space="PSUM") as psum1,
        ):

            # ---- persistent tiles -------------------------------------
            wk_sb = singles.tile([P, DC, F], f32)
            dk = nc.sync.dma_start(wk_sb, wk_d[:, :, :].rearrange("d p f -> p d f"))
            wq_sb = singles.tile([P, DC, F], f32)
            dq = nc.sync.dma_start(wq_sb, wq_d[:, :, :].rearrange("d p f -> p d f"))
            wv_sb = singles.tile([P, DC, F], f32)
            dv = nc.sync.dma_start(wv_sb, wv_d[:, :, :].rearrange("d p f -> p d f"))
            bkt_sb = singles.tile([P, FC], f32)
            nc.sync.dma_start(bkt_sb, bkt_d[:, :])
            bqt_sb = singles.tile([P, FC], f32)
            nc.sync.dma_start(bqt_sb, bqt_d[:, :])
            bvb_sb = singles.tile([P, F], f32)
            nc.gpsimd.dma_start(bvb_sb, bvb_d[:, :].to_broadcast([P, F]))
            ident = singles.tile([P, P], f32)
            nc.gpsimd.memset(ident, 0.0)
            nc.gpsimd.affine_select(
                out=ident, in_=ident,
                compare_op=mybir.AluOpType.not_equal, fill=1.0, base=0,
                pattern=[[-1, P]], channel_multiplier=1)
            scratch_ps = psum1.tile([P, P], f32)

            def pe_touch(*aps):
                # A real (tiny) PE transpose per AP: the PE observes each
                # producer's semaphore here, so following matmuls — whose
                # fp32 LDWEIGHTS struct holds only ONE sync-wait command —
                # never need more than one wait condition. Output goes to a
                # dedicated scratch PSUM slot nothing reads (PE-only WAW).
                for ap in aps:
                    nc.tensor.transpose(scratch_ps, ap[:, :1, :P] if len(ap.shape) == 3 else ap[:, :P], ident)

            nc.tensor.transpose(scratch_ps, ident, ident)  # observe Pool sem
            pe_touch(wk_sb, wq_sb, wv_sb)

            kT = singles.tile([P, FC, T], f32)       # K^T: [kfeat, s]
            vN = singles.tile([P, NSTRIP, F], f32)   # V:   [s, vfeat] (no bias)
            qT = singles.tile([P, FC, NOWN * P], f32)  # Q^T: [kfeat, own t]

            # ---- phase A: projections for one 256-col t-chunk ----------
            def proj_chunk(m):
                t0 = m * 256
                xc = xin.tile([P, DC, 256], f32, tag="xc")
                nc.sync.dma_start(
                    xc, xT[:, :, t0:t0 + 256].rearrange("d p t -> p d t"))
                xqc = xin.tile([P, DC, P], f32, tag="xqc")
                nc.sync.dma_start(
                    xqc, xqT[:, :, m * P:(m + 1) * P].rearrange("d p t -> p d t"))
                pe_touch(xc, xqc)

                # K^T[:, fc, t0:t0+256]
                for fc in range(FC):
                    ps = psum.tile([P, F], f32, tag="ps", name=f"ps_k_{m}_{fc}")
                    for dc in range(DC):
                        nc.tensor.matmul(
                            ps[:, :256],
                            lhsT=wk_sb[:, dc, fc * P:(fc + 1) * P],
                            rhs=xc[:, dc, :],
                            start=(dc == 0), stop=(dc == DC - 1))
                    nc.vector.tensor_scalar_add(
                        kT[:, fc, t0:t0 + 256], ps[:, :256], bkt_sb[:, fc:fc + 1])

                # V strips 2m, 2m+1 (bias folded into output stage)
                for st in range(2):
                    s = 2 * m + st
                    ps = psum.tile([P, F], f32, tag="ps", name=f"ps_v_{m}_{st}")
                    for dc in range(DC):
                        nc.tensor.matmul(
                            ps,
                            lhsT=xc[:, dc, st * P:(st + 1) * P],
                            rhs=wv_sb[:, dc, :],
                            start=(dc == 0), stop=(dc == DC - 1))
                    nc.vector.tensor_copy(vN[:, s, :], ps)

                # Q^T[:, fc, m*128:(m+1)*128] (own strip j = m)
                for fc in range(FC):
                    ps = psum.tile([P, F], f32, tag="ps", name=f"ps_q_{m}_{fc}")
                    for dc in range(DC):
                        nc.tensor.matmul(
                            ps[:, :P],
                            lhsT=wq_sb[:, dc, fc * P:(fc + 1) * P],
                            rhs=xqc[:, dc, :],
                            start=(dc == 0), stop=(dc == DC - 1))
                    nc.vector.tensor_scalar_add(
                        qT[:, fc, m * P:(m + 1) * P], ps[:, :P], bqt_sb[:, fc:fc + 1])

            # ---- phase B: attention for own strip j --------------------
            def attn_strip(j):
                nk = NK[j]
                nkeys = nk * F
                nsc = nkeys // P   # 128-wide key chunks

                L = work.tile([P, 4 * F], f32, tag="lp", name=f"L_{j}")
                for k in range(nk):
                    ps = psum.tile([P, F], f32, tag="ps", name=f"ps_l_{j}_{k}")
                    for fc in range(FC):
                        nc.tensor.matmul(
                            ps,
                            lhsT=qT[:, fc, j * P:(j + 1) * P],
                            rhs=kT[:, fc, k * F:(k + 1) * F],
                            start=(fc == 0), stop=(fc == FC - 1))
                    if k == nk - 1:
                        msk = small.tile([P, F], f32, tag="msk", name=f"msk_{j}")
                        nc.sync.dma_start(msk, mask_d[j])
                        nc.vector.tensor_add(L[:, k * F:(k + 1) * F], ps, msk)
                    else:
                        nc.vector.tensor_copy(L[:, k * F:(k + 1) * F], ps)

                negmax = small.tile([P, 1], f32, tag="negmax", name=f"negmax_{j}")
                nc.vector.tensor_reduce(
                    negmax, L[:, :nkeys], axis=mybir.AxisListType.X,
                    op=mybir.AluOpType.max, negate=True)

                E = work.tile([P, 4 * F], f32, tag="lp", name=f"E_{j}")
                rowsum = small.tile([P, 1], f32, tag="rowsum", name=f"rowsum_{j}")
                nc.scalar.activation(
                    out=E[:, :nkeys], in_=L[:, :nkeys],
                    func=mybir.ActivationFunctionType.Exp,
                    bias=negmax, scale=1.0, accum_out=rowsum)

                rinv = small.tile([P, 1], f32, tag="rinv", name=f"rinv_{j}")
                nc.vector.tensor_scalar_mul(rowsum, rowsum, SQRT_K)
                nc.vector.reciprocal(rinv, rowsum)

                pT = work.tile([P, 4 * F], f32, tag="pt", name=f"pT_{j}")
                pe_touch(E)
                for sc in range(nsc):
                    pst = psum.tile([P, F], f32, tag="ps", name=f"ps_t_{j}_{sc}")
                    nc.tensor.transpose(
                        pst[:, :P], E[:, sc * P:(sc + 1) * P], ident)
                    nc.vector.tensor_copy(pT[:, sc * P:(sc + 1) * P], pst[:, :P])

                po = psum.tile([P, F], f32, tag="ps", name=f"ps_o_{j}")
                for sc in range(nsc):
                    nc.tensor.matmul(
                        po,
                        lhsT=pT[:, sc * P:(sc + 1) * P],
                        rhs=vN[:, sc, :],
                        start=(sc == 0), stop=(sc == nsc - 1))

                ob = small.tile([P, F], f32, tag="ob", name=f"ob_{j}")
                nc.vector.scalar_tensor_tensor(
                    out=ob, in0=po, scalar=rinv, in1=bvb_sb,
                    op0=mybir.AluOpType.mult, op1=mybir.AluOpType.add)
                nc.sync.dma_start(out_d[j], ob)

            # Interleave projection chunks and attention strips so the PE
            # stream stays dense and attention starts as soon as its keys
            # are projected. attn j needs key chunks 0..2*NK[j]-1 and qT[j]
            # (produced by proj chunk j).
            proj_chunk(0)
            proj_chunk(1)
            attn_strip(0)
            attn_strip(1)
            proj_chunk(2)
            proj_chunk(3)
            attn_strip(2)
            attn_strip(3)
            proj_chunk(4)
            proj_chunk(5)
            attn_strip(4)
            attn_strip(5)
            proj_chunk(6)
            proj_chunk(7)
            attn_strip(6)
            attn_strip(7)

    _split_multi_waits(nc)
    return nc


_NO_SPLIT_OPCODES = {"CollectiveCompute", "EventSemaphore"}


def _split_multi_waits(nc):
    """Every TPB instruction carries exactly ONE sync-wait and ONE sync-update
    slot (NEURON_ISA_TPB_EVENTS). Tile sometimes emits more; walrus codegen
    then fails with "Too many sync wait commands". Split the extras onto
    adjacent same-engine NoOps (waits before, updates after)."""
    for f in nc.m.functions:
        for bb in f.blocks:
            new = []
            changed = False
            for inst in bb.instructions:
                si = inst.sync_info
                if si is None or inst.opcode in _NO_SPLIT_OPCODES:
                    new.append(inst)
                    continue
                waits = list(si.on_wait or [])
                ups = list(si.on_update or [])
                if len(waits) > 1:
                    for k, w in enumerate(waits[:-1]):
                        nop = mybir.InstNoOp(
                            name=f"{inst.name}-sw{k}", engine=inst.engine,
                            ins=[], outs=[],
                            sync_info=mybir.SyncInfo(on_wait=[w], on_update=[]))
                        nc.register_instruction(nop, overwrite=True)
                        new.append(nop)
                    si.on_wait = [waits[-1]]
                    changed = True
                new.append(inst)
                if len(ups) > 1:
                    si.on_update = [ups[0]]
                    for k, u in enumerate(ups[1:]):
                        nop = mybir.InstNoOp(
                            name=f"{inst.name}-su{k}", engine=inst.engine,
                            ins=[], outs=[],
                            sync_info=mybir.SyncInfo(on_wait=[], on_update=[u]))
                        nc.register_instruction(nop, overwrite=True)
                        new.append(nop)
                    changed = True
            if changed:
                bb.instructions = new


_nc_cache = None


def _get_nc():
    global _nc_cache
    if _nc_cache is None:
        _nc_cache = _build_bass()
    return _nc_cache


def _make_masks(c: int) -> np.ndarray:
    """Boundary-key-tile additive masks for parity c, all own strips."""
    m = np.zeros((NOWN, P, F), dtype=np.float32)
    r = np.arange(P)[:, None]
    sl = np.arange(F)[None, :]
    for j in range(NOWN):
        t0 = (2 * j + c) * P
        base = F * (NK[j] - 1)
        m[j] = np.where(base + sl <= t0 + r, 0.0, NEG)
    return m


def build_in_maps(x, Wk, bk, Wq, bq, Wv, bv):
    Wk = np.asarray(Wk, np.float32); bk = np.asarray(bk, np.float32)
    Wq = np.asarray(Wq, np.float32); bq = np.asarray(bq, np.float32)
    Wv = np.asarray(Wv, np.float32); bv = np.asarray(bv, np.float32)

    wk_r = np.ascontiguousarray(Wk.reshape(DC, P, F))
    wq_r = np.ascontiguousarray(Wq.reshape(DC, P, F))
    wv_r = np.ascontiguousarray(Wv.reshape(DC, P, F))
    bkt = np.ascontiguousarray(bk.reshape(FC, P).T)
    bqt = np.ascontiguousarray(bq.reshape(FC, P).T)
    bvb = np.ascontiguousarray((bv / SQRT_K).reshape(1, F))
    masks = [_make_masks(0), _make_masks(1)]

    in_maps = []
    for core in range(8):
        b, c = divmod(core, 2)
        xb = x[b]                                    # [T, D]
        xTb = np.ascontiguousarray(xb.T.reshape(DC, P, T))
        xq = xb.reshape(NSTRIP, P, D)[c::2]           # own strips' rows
        xq = xq.reshape(NOWN * P, D)
        xqTb = np.ascontiguousarray(xq.T.reshape(DC, P, NOWN * P))
        in_maps.append({
            "xt": xTb, "xqt": xqTb,
            "wk": wk_r, "wq": wq_r, "wv": wv_r,
            "bkt": bkt, "bqt": bqt, "bvb": bvb,
            "mask": masks[c],
        })
    return in_maps


def kernel(minibatch, Wk, bk, Wq, bq, Wv, bv):
    global last_result
    x = np.ascontiguousarray(np.asarray(minibatch, dtype=np.float32))
    in_maps = build_in_maps(x, Wk, bk, Wq, bq, Wv, bv)
    nc = _get_nc()
    res = run_bass_kernel_spmd(nc, in_maps, core_ids=list(range(8)))
    last_result = res

    out = np.empty((B, T, D + F), dtype=np.float32)
    out[:, :, :D] = x
    for core in range(8):
        b, c = divmod(core, 2)
        r = res.results[core]["out"]                 # [NOWN, P, F]
        for j in range(NOWN):
            s = 2 * j + c
            out[b, s * P:(s + 1) * P, D:] = r[j]
    return out


# revision 17
# speedup vs baseline: 433.1498x; 433.1498x over previous
"""Trainium2 Bass kernel for causal AttentionBlock.

Reference computation (per batch b):
    K = x @ Wk + bk ; Q = x @ Wq + bq ; V = x @ Wv + bv      # [T, 512]
    logits = Q @ K^T  (causal: allow s <= t)
    probs  = softmax(logits) / sqrt(512)
    read   = probs @ V
    out    = concat([x, read], axis=-1)                      # [T, 1536]

Shapes: B=4, T=2048, D=1024, K=V=512, all float32.

Sharding over 8 cores: core = 2*b + c where b = batch, c = query parity.
Core (b, c) owns query strips s = 2j + c (j = 0..7), 128 rows each —
interleaved strips balance causal work exactly: the number of 512-wide key
tiles needed per own-strip j is NK[j] = [1,1,2,2,3,3,4,4] for BOTH parities,
so a single SPMD program serves all cores.  Per-core differences (which
rows are queries, causal boundary masks) are carried in the input data.

Each core computes K^T and V over the full T (duplicated within the batch
pair), Q^T for its own 1024 rows, then causally-truncated attention.
The sqrt(512) quirk and bv are folded into the output stage:
    read = (exp @ V0) * rinv + bv/sqrt(512),  rinv = 1/(rowsum * sqrt(512))
because softmax rows sum to 1 (so the bv term picks up exactly 1/sqrt(512)).
"""

import os
os.environ.setdefault("JAX_COMPILATION_CACHE_DIR", "/tmp/jaxcache")

import numpy as np

import concourse.bass as bass
import concourse.tile as tile
from concourse import mybir
from concourse.bass_utils import run_bass_kernel_spmd
from concourse.masks import make_identity
from concourse.tile import add_dep_helper

P = 128
B, T, D, F = 4, 2048, 1024, 512
DC = D // P        # 8 contraction chunks
FC = F // P        # 4 feature chunks
NSTRIP = T // P    # 16 strips per batch
NOWN = NSTRIP // 2  # 8 own strips per core
NK = [1, 1, 2, 2, 3, 3, 4, 4]   # 512-wide key tiles per own strip (both parities)
SQRT_K = float(np.sqrt(512.0))
NEG = -1.0e30

f32 = mybir.dt.float32

last_result = None  # BassKernelResults of the most recent run (for test.py)


def _build_bass(repeat: int = 1) -> bass.Bass:
    nc = bass.Bass()

    xT = nc.dram_tensor("xt", [DC, P, T], f32, kind="ExternalInput")
    xqT = nc.dram_tensor("xqt", [DC, P, NOWN * P], f32, kind="ExternalInput")
    wk_d = nc.dram_tensor("wk", [DC, P, F], f32, kind="ExternalInput")
    wq_d = nc.dram_tensor("wq", [DC, P, F], f32, kind="ExternalInput")
    wv_d = nc.dram_tensor("wv", [DC, P, F], f32, kind="ExternalInput")
    bkt_d = nc.dram_tensor("bkt", [P, FC], f32, kind="ExternalInput")
    bqt_d = nc.dram_tensor("bqt", [P, FC], f32, kind="ExternalInput")
    bvb_d = nc.dram_tensor("bvb", [1, F], f32, kind="ExternalInput")
    mask_d = nc.dram_tensor("mask", [NOWN, P, F], f32, kind="ExternalInput")
    out_d = nc.dram_tensor("out", [NOWN, P, F], f32, kind="ExternalOutput")

    with tile.TileContext(nc) as tc:
        with (
            tc.tile_pool(name="singles", bufs=1) as singles,
            tc.tile_pool(name="xin", bufs=2) as xin,
            tc.tile_pool(name="work", bufs=2) as work,
            tc.tile_pool(name="small", bufs=4) as small,
            tc.tile_pool(name="psum", bufs=7, space="PSUM") as psum,
            tc.tile_pool(name="psum1", bufs=1, space="PSUM") as psum1,
        ):

            # ---- persistent tiles -------------------------------------
            wk_sb = singles.tile([P, DC, F], f32)
            dk = nc.sync.dma_start(wk_sb, wk_d[:, :, :].rearrange("d p f -> p d f"))
            wq_sb = singles.tile([P, DC, F], f32)
            dq = nc.sync.dma_start(wq_sb, wq_d[:, :, :].rearrange("d p f -> p d f"))
            wv_sb = singles.tile([P, DC, F], f32)
            dv = nc.sync.dma_start(wv_sb, wv_d[:, :, :].rearrange("d p f -> p d f"))
            bkt_sb = singles.tile([P, FC], f32)
            nc.sync.dma_start(bkt_sb, bkt_d[:, :])
            bqt_sb = singles.tile([P, FC], f32)
            nc.sync.dma_start(bqt_sb, bqt_d[:, :])
            bvb_sb = singles.tile([P, F], f32)
            nc.gpsimd.dma_start(bvb_sb, bvb_d[:, :].to_broadcast([P, F]))
            ident = singles.tile([P, P], f32)
            nc.gpsimd.memset(ident, 0.0)
            nc.gpsimd.affine_select(
                out=ident, in_=ident,
                compare_op=mybir.AluOpType.not_equal, fill=1.0, base=0,
                pattern=[[-1, P]], channel_multiplier=1)
            scratch_ps = psum1.tile([P, P], f32)

            def pe_touch(*aps):
                # A real (tiny) PE transpose per AP: the PE observes each
                # producer's semaphore here, so following matmuls — whose
                # fp32 LDWEIGHTS struct holds only ONE sync-wait command —
                # never need more than one wait condition. Output goes to a
                # dedicated scratch PSUM slot nothing reads (PE-only WAW).
                for ap in aps:
                    nc.tensor.transpose(scratch_ps, ap[:, :1, :P] if len(ap.shape) == 3 else ap[:, :P], ident)

            nc.tensor.transpose(scratch_ps, ident, ident)  # observe Pool sem
            pe_touch(wk_sb, wq_sb, wv_sb)

            kT = singles.tile([P, FC, T], f32)       # K^T: [kfeat, s]
            vN = singles.tile([P, NSTRIP, F], f32)   # V:   [s, vfeat] (no bias)
            qT = singles.tile([P, FC, NOWN * P], f32)  # Q^T: [kfeat, own t]

            # ---- phase A: projections for one 256-col t-chunk ----------
            def proj_chunk(m):
                t0 = m * 256
                xc = xin.tile([P, DC, 256], f32, tag="xc")
                nc.sync.dma_start(
                    xc, xT[:, :, t0:t0 + 256].rearrange("d p t -> p d t"))
                xqc = xin.tile([P, DC, P], f32, tag="xqc")
                nc.sync.dma_start(
                    xqc, xqT[:, :, m * P:(m + 1) * P].rearrange("d p t -> p d t"))
                pe_touch(xc, xqc)

                # K^T[:, fc, t0:t0+256]
                for fc in range(FC):
                    ps = psum.tile([P, F], f32, tag="ps", name=f"ps_k_{m}_{fc}")
                    for dc in range(DC):
                        nc.tensor.matmul(
                            ps[:, :256],
                            lhsT=wk_sb[:, dc, fc * P:(fc + 1) * P],
                            rhs=xc[:, dc, :],
                            start=(dc == 0), stop=(dc == DC - 1))
                    nc.vector.tensor_scalar_add(
                        kT[:, fc, t0:t0 + 256], ps[:, :256], bkt_sb[:, fc:fc + 1])

                # V strips 2m, 2m+1 (bias folded into output stage)
                for st in range(2):
                    s = 2 * m + st
                    ps = psum.tile([P, F], f32, tag="ps", name=f"ps_v_{m}_{st}")
                    for dc in range(DC):
                        nc.tensor.matmul(
                            ps,
                            lhsT=xc[:, dc, st * P:(st + 1) * P],
                            rhs=wv_sb[:, dc, :],
                            start=(dc == 0), stop=(dc == DC - 1))
                    nc.vector.tensor_copy(vN[:, s, :], ps)

                # Q^T[:, fc, m*128:(m+1)*128] (own strip j = m)
                for fc in range(FC):
                    ps = psum.tile([P, F], f32, tag="ps", name=f"ps_q_{m}_{fc}")
                    for dc in range(DC):
                        nc.tensor.matmul(
                            ps[:, :P],
                            lhsT=wq_sb[:, dc, fc * P:(fc + 1) * P],
                            rhs=xqc[:, dc, :],
                            start=(dc == 0), stop=(dc == DC - 1))
                    nc.vector.tensor_scalar_add(
                        qT[:, fc, m * P:(m + 1) * P], ps[:, :P], bqt_sb[:, fc:fc + 1])

            # ---- phase B: attention for own strip j --------------------
            def attn_strip(j):
                nk = NK[j]
                nkeys = nk * F
                nsc = nkeys // P   # 128-wide key chunks

                L = work.tile([P, 4 * F], f32, tag="lp", name=f"L_{j}")
                for k in range(nk):
                    ps = psum.tile([P, F], f32, tag="ps", name=f"ps_l_{j}_{k}")
                    for fc in range(FC):
                        nc.tensor.matmul(
                            ps,
                            lhsT=qT[:, fc, j * P:(j + 1) * P],
                            rhs=kT[:, fc, k * F:(k + 1) * F],
                            start=(fc == 0), stop=(fc == FC - 1))
                    if k == nk - 1:
                        msk = small.tile([P, F], f32, tag="msk", name=f"msk_{j}")
                        nc.sync.dma_start(msk, mask_d[j])
                        nc.vector.tensor_add(L[:, k * F:(k + 1) * F], ps, msk)
                    else:
                        nc.vector.tensor_copy(L[:, k * F:(k + 1) * F], ps)

                negmax = small.tile([P, 1], f32, tag="negmax", name=f"negmax_{j}")
                nc.vector.tensor_reduce(
                    negmax, L[:, :nkeys], axis=mybir.AxisListType.X,
                    op=mybir.AluOpType.max, negate=True)

                E = work.tile([P, 4 * F], f32, tag="lp", name=f"E_{j}")
                rowsum = small.tile([P, 1], f32, tag="rowsum", name=f"rowsum_{j}")
                nc.scalar.activation(
                    out=E[:, :nkeys], in_=L[:, :nkeys],
                    func=mybir.ActivationFunctionType.Exp,
                    bias=negmax, scale=1.0, accum_out=rowsum)

                rinv = small.tile([P, 1], f32, tag="rinv", name=f"rinv_{j}")
                nc.vector.tensor_scalar_mul(rowsum, rowsum, SQRT_K)
                nc.vector.reciprocal(rinv, rowsum)

                pT = work.tile([P, 4 * F], f32, tag="pt", name=f"pT_{j}")
                pe_touch(E)
                for sc in range(nsc):
                    pst = psum.tile([P, F], f32, tag="ps", name=f"ps_t_{j}_{sc}")
                    nc.tensor.transpose(
                        pst[:, :P], E[:, sc * P:(sc + 1) * P], ident)
                    nc.vector.tensor_copy(pT[:, sc * P:(sc + 1) * P], pst[:, :P])

                po = psum.tile([P, F], f32, tag="ps", name=f"ps_o_{j}")
                for sc in range(nsc):
                    nc.tensor.matmul(
                        po,
                        lhsT=pT[:, sc * P:(sc + 1) * P],
                        rhs=vN[:, sc, :],
                        start=(sc == 0), stop=(sc == nsc - 1))

                ob = small.tile([P, F], f32, tag="ob", name=f"ob_{j}")
                nc.vector.scalar_tensor_tensor(
                    out=ob, in0=po, scalar=rinv, in1=bvb_sb,
                    op0=mybir.AluOpType.mult, op1=mybir.AluOpType.add)
                nc.sync.dma_start(out_d[j], ob)

            # Interleave projection chunks and attention strips so the PE
            # stream stays dense and attention starts as soon as its keys
            # are projected. attn j needs key chunks 0..2*NK[j]-1 and qT[j]
            # (produced by proj chunk j).
            for _rep in range(repeat):
                for mm2 in range(0, NOWN, 2):
                    proj_chunk(mm2)
                    proj_chunk(mm2 + 1)
                    attn_strip(mm2)
                    attn_strip(mm2 + 1)

    _split_multi_waits(nc)
    return nc


_NO_SPLIT_OPCODES = {"CollectiveCompute", "EventSemaphore"}


def _split_multi_waits(nc):
    """Every TPB instruction carries exactly ONE sync-wait and ONE sync-update
    slot (NEURON_ISA_TPB_EVENTS). Tile sometimes emits more; walrus codegen
    then fails with "Too many sync wait commands". Split the extras onto
    adjacent same-engine NoOps (waits before, updates after)."""
    for f in nc.m.functions:
        for bb in f.blocks:
            new = []
            changed = False
            for inst in bb.instructions:
                si = inst.sync_info
                if si is None or inst.opcode in _NO_SPLIT_OPCODES:
                    new.append(inst)
                    continue
                waits = list(si.on_wait or [])
                ups = list(si.on_update or [])
                if len(waits) > 1:
                    for k, w in enumerate(waits[:-1]):
                        nop = mybir.InstNoOp(
                            name=f"{inst.name}-sw{k}", engine=inst.engine,
                            ins=[], outs=[],
                            sync_info=mybir.SyncInfo(on_wait=[w], on_update=[]))
                        nc.register_instruction(nop, overwrite=True)
                        new.append(nop)
                    si.on_wait = [waits[-1]]
                    changed = True
                new.append(inst)
                if len(ups) > 1:
                    si.on_update = [ups[0]]
                    for k, u in enumerate(ups[1:]):
                        nop = mybir.InstNoOp(
                            name=f"{inst.name}-su{k}", engine=inst.engine,
                            ins=[], outs=[],
                            sync_info=mybir.SyncInfo(on_wait=[], on_update=[u]))
                        nc.register_instruction(nop, overwrite=True)
                        new.append(nop)
                    changed = True
            if changed:
                bb.instructions = new


_nc_cache = None


def _get_nc():
    global _nc_cache
    if _nc_cache is None:
        _nc_cache = _build_bass()
    return _nc_cache


def _make_masks(c: int) -> np.ndarray:
    """Boundary-key-tile additive masks for parity c, all own strips."""
    m = np.zeros((NOWN, P, F), dtype=np.float32)
    r = np.arange(P)[:, None]
    sl = np.arange(F)[None, :]
    for j in range(NOWN):
        t0 = (2 * j + c) * P
        base = F * (NK[j] - 1)
        m[j] = np.where(base + sl <= t0 + r, 0.0, NEG)
    return m


def build_in_maps(x, Wk, bk, Wq, bq, Wv, bv):
    Wk = np.asarray(Wk, np.float32); bk = np.asarray(bk, np.float32)
    Wq = np.asarray(Wq, np.float32); bq = np.asarray(bq, np.float32)
    Wv = np.asarray(Wv, np.float32); bv = np.asarray(bv, np.float32)

    wk_r = np.ascontiguousarray(Wk.reshape(DC, P, F))
    wq_r = np.ascontiguousarray(Wq.reshape(DC, P, F))
    wv_r = np.ascontiguousarray(Wv.reshape(DC, P, F))
    bkt = np.ascontiguousarray(bk.reshape(FC, P).T)
    bqt = np.ascontiguousarray(bq.reshape(FC, P).T)
    bvb = np.ascontiguousarray((bv / SQRT_K).reshape(1, F))
    masks = [_make_masks(0), _make_masks(1)]

    in_maps = []
    for core in range(8):
        b, c = divmod(core, 2)
        xb = x[b]                                    # [T, D]
        xTb = np.ascontiguousarray(xb.T.reshape(DC, P, T))
        xq = xb.reshape(NSTRIP, P, D)[c::2]           # own strips' rows
        xq = xq.reshape(NOWN * P, D)
        xqTb = np.ascontiguousarray(xq.T.reshape(DC, P, NOWN * P))
        in_maps.append({
            "xt": xTb, "xqt": xqTb,
            "wk": wk_r, "wq": wq_r, "wv": wv_r,
            "bkt": bkt, "bqt": bqt, "bvb": bvb,
            "mask": masks[c],
        })
    return in_maps


def kernel(minibatch, Wk, bk, Wq, bq, Wv, bv):
    global last_result
    x = np.ascontiguousarray(np.asarray(minibatch, dtype=np.float32))
    in_maps = build_in_maps(x, Wk, bk, Wq, bq, Wv, bv)
    nc = _get_nc()
    res = run_bass_kernel_spmd(nc, in_maps, core_ids=list(range(8)))
    last_result = res

    out = np.empty((B, T, D + F), dtype=np.float32)
    out[:, :, :D] = x
    for core in range(8):
        b, c = divmod(core, 2)
        r = res.results[core]["out"]                 # [NOWN, P, F]
        for j in range(NOWN):
            s = 2 * j + c
            out[b, s * P:(s + 1) * P, D:] = r[j]
    return out
